# revision 8
# baseline (speedup 1.0000x reference)
"""Sliding-window attention block (B=4, S=2048, E=1024, H=16, D=64,
window_left=512, window_right=0) on 8 Trainium2 NeuronCores.

Sharding: core c handles batch b=c//2 and head group g=c%2 (8 heads each).
Each core computes qkv projection for its heads over the full sequence,
banded attention (256-query stripes, 128-key blocks), and a partial output
projection; the host sums the two head-group partials per batch.

All device dataflow is feature-major (transposed): qkT/attnT/outT are
[features, seq].  Window masking is added into the scores PSUM with
identity-weight matmuls of precomputed -30000 bias tiles.  Key padding
(j >= seq_len) is handled by zeroing V rows and the denominator-ones
column, so padded keys drop out of both numerator and denominator.
Fully-masked query rows (i >= seq_len+512) are fixed up on the host
(reference semantics: uniform attention over all keys).
"""

import numpy as np

B, S, E, H, D = 4, 2048, 1024, 16, 64
NCORES = 8
HPC = H // 2          # heads per core
WIN = 512             # window_left (window_right = 0)
NEG = -30000.0
NQ = 256              # query stripe width
NST = S // NQ         # stripes
SCALE = 1.0 / np.sqrt(np.float32(D))

_cache = {}


def _build_program():
    from contextlib import ExitStack

    import concourse.bass as bass  # noqa: F401
    import concourse.mybir as mybir
    import concourse.tile as tile
    from concourse import bacc

    dt = mybir.dt
    f32, f32r = dt.float32, dt.float32r
    AF = mybir.ActivationFunctionType
    mult = mybir.AluOpType.mult

    nc = bacc.Bacc("TRN2", target_bir_lowering=False, debug=False,
                   num_devices=NCORES)

    xT = nc.dram_tensor("xT", [E, S], f32r, kind="ExternalInput")
    wqk = nc.dram_tensor("wqk", [E, 2 * HPC * D], f32r, kind="ExternalInput")
    wv = nc.dram_tensor("wv", [E, HPC * D], f32r, kind="ExternalInput")
    wo = nc.dram_tensor("wo", [HPC * D, E], f32r, kind="ExternalInput")
    vmask = nc.dram_tensor("vmask", [128, 16], f32, kind="ExternalInput")
    vone8 = nc.dram_tensor("vone8", [S, HPC], f32r, kind="ExternalInput")
    masks = nc.dram_tensor("masks", [3, 128, 512], f32r, kind="ExternalInput")
    outT = nc.dram_tensor("outT", [E, S], f32, kind="ExternalOutput")

    with tile.TileContext(nc) as tc, ExitStack() as ctx:
        persist = ctx.enter_context(tc.tile_pool(name="persist", bufs=1))

        qkT = [persist.tile([128, S], f32r, name=f"qkT{i}", tag=f"qkT{i}") for i in range(8)]
        vsb = [persist.tile([128, HPC, D + 1], f32r, name=f"v{t}", tag=f"v{t}")
               for t in range(16)]
        maskAB = persist.tile([128, 512], f32r, tag="maskAB")
        maskCD = persist.tile([128, 512], f32r, tag="maskCD")
        vmsb = persist.tile([128, 16], f32, tag="vmsb")
        ident = persist.tile([128, 128], f32r, tag="ident")

        nc.sync.dma_start(out=ident, in_=masks[2, :, 0:128])
        nc.sync.dma_start(out=maskAB, in_=masks[0])
        nc.sync.dma_start(out=maskCD, in_=masks[1])
        nc.sync.dma_start(out=vmsb, in_=vmask[:, :])

        # ---- phase 1+2: qk projection (feature-major) + V (seq-major) ----
        with tc.tile_pool(name="wgt12", bufs=1) as wpool, \
             tc.tile_pool(name="xc", bufs=2) as xpool, \
             tc.tile_pool(name="qkps", bufs=2, space="PSUM") as qkps, \
             tc.tile_pool(name="vps", bufs=2, space="PSUM") as vps:
            wqk_sb = [wpool.tile([128, 2 * HPC * D], f32r, name=f"wqk{k}",
                                 tag=f"wqk{k}") for k in range(8)]
            wv_sb = [wpool.tile([128, HPC * D], f32r, name=f"wv{k}",
                                tag=f"wv{k}") for k in range(8)]
            for k in range(8):
                nc.sync.dma_start(out=wqk_sb[k],
                                  in_=wqk[k * 128:(k + 1) * 128, :])
                nc.sync.dma_start(out=wv_sb[k],
                                  in_=wv[k * 128:(k + 1) * 128, :])
            for nb in range(4):
                xc = [xpool.tile([128, 512], f32r, name=f"xc{k}", tag=f"xc{k}")
                      for k in range(8)]
                for k in range(8):
                    nc.sync.dma_start(
                        out=xc[k],
                        in_=xT[k * 128:(k + 1) * 128, nb * 512:(nb + 1) * 512])
                for mb in range(8):
                    ps = qkps.tile([128, 512], f32, tag="qk")
                    for k in range(8):
                        nc.tensor.matmul(
                            ps[:, :],
                            lhsT=wqk_sb[k][:, mb * 128:(mb + 1) * 128],
                            rhs=xc[k][:, :],
                            start=(k == 0), stop=(k == 7))
                    dst = qkT[mb][:, nb * 512:(nb + 1) * 512]
                    if mb % 2 == 0:
                        nc.scalar.copy(dst, ps[:, :])
                    else:
                        nc.vector.tensor_copy(dst, ps[:, :])
                for t4 in range(4):
                    t = nb * 4 + t4
                    ps = vps.tile([128, 512], f32, tag="v")
                    for k in range(8):
                        nc.tensor.matmul(
                            ps[:, :],
                            lhsT=xc[k][:, t4 * 128:(t4 + 1) * 128],
                            rhs=wv_sb[k][:, :],
                            start=(k == 0), stop=(k == 7))
                    nc.vector.tensor_scalar(
                        out=vsb[t][:, :, 0:D],
                        in0=ps.rearrange("p (h d) -> p h d", h=HPC),
                        scalar1=vmsb[:, t:t + 1],
                        scalar2=None,
                        op0=mult)
                    nc.sync.dma_start(out=vsb[t][:, :, D],
                                      in_=vone8[t * 128:(t + 1) * 128, :])

        # ---- phase 3: banded attention ----
        apool = ctx.enter_context(tc.tile_pool(name="attn", bufs=1))
        attnT = [apool.tile([128, S], f32r, name=f"attnT{i}", tag=f"attnT{i}")
                 for i in range(4)]
        with tc.tile_pool(name="scps", bufs=2, space="PSUM") as spool, \
             tc.tile_pool(name="ops", bufs=2, space="PSUM") as opool, \
             tc.tile_pool(name="expT", bufs=3) as epool, \
             tc.tile_pool(name="rc", bufs=4) as rpool, \
             tc.tile_pool(name="rb", bufs=4) as rbpool:
            for s in range(NST):
                if s == 0:
                    kbs, mask_regions = [4, 5], [(0, maskCD)]
                elif s == 1:
                    kbs, mask_regions = [2, 3, 4, 5], [(2, maskCD)]
                else:
                    kbs, mask_regions = [0, 1, 2, 3, 4, 5], [(0, maskAB),
                                                             (4, maskCD)]
                nkb = len(kbs)
                base_kt = 2 * s - 4
                # per-PSUM-bank op lists: regions 2b, 2b+1 share a bank and
                # must form one accumulation group (start clears the bank)
                banks = []
                for b0 in range(0, nkb, 2):
                    ops = [("score", b0), ("score", b0 + 1)]
                    for reg0, mk in mask_regions:
                        if reg0 == b0:
                            ops += [("mask", (b0, 0, mk)), ("mask", (b0 + 1, 1, mk))]
                    banks.append(ops)
                for h in range(HPC):
                    po = (h % 2) * 64
                    qt = qkT[h // 2]
                    kt_ = qkT[4 + h // 2]
                    sc = spool.tile([128, 6, NQ], f32, tag="sc")
                    for ops in banks:
                        for oi, (kind, arg) in enumerate(ops):
                            first, last = oi == 0, oi == len(ops) - 1
                            if kind == "score":
                                ktile = base_kt + kbs[arg]
                                nc.tensor.matmul(
                                    sc[:, arg, :],
                                    lhsT=kt_[po:po + 64,
                                               ktile * 128:(ktile + 1) * 128],
                                    rhs=qt[po:po + 64, s * NQ:(s + 1) * NQ],
                                    start=first, stop=last)
                            else:
                                reg, j, mk = arg
                                nc.tensor.matmul(
                                    sc[:, reg, :],
                                    lhsT=ident[:, :],
                                    rhs=mk[:, j * 256:(j + 1) * 256],
                                    start=first, stop=last)
                    ex = epool.tile([128, 6, NQ], f32r, tag="ex")
                    nc.scalar.activation(ex[:, 0:nkb, :], sc[:, 0:nkb, :],
                                         AF.Exp)
                    ot = opool.tile([D + 1, NQ], f32, tag="ot")
                    for i, kb in enumerate(kbs):
                        ktile = base_kt + kb
                        nc.tensor.matmul(
                            ot[:, :],
                            lhsT=vsb[ktile][:, h, :],
                            rhs=ex[:, i, :],
                            start=(i == 0), stop=(i == nkb - 1))
                    rc = rpool.tile([1, NQ], f32, tag="rc")
                    nc.vector.reciprocal(rc[:, :], ot[D:D + 1, :])
                    rb = rbpool.tile([64, NQ], f32, tag="rb")
                    nc.gpsimd.partition_broadcast(rb[:, :], rc[:, :])
                    nc.vector.tensor_tensor(
                        out=attnT[h // 2][po:po + 64, s * NQ:(s + 1) * NQ],
                        in0=ot[0:D, :], in1=rb[:, :], op=mult)

        # ---- phase 4: output projection ----
        with tc.tile_pool(name="wo4", bufs=1) as wopool, \
             tc.tile_pool(name="oprj", bufs=4, space="PSUM") as ppool, \
             tc.tile_pool(name="ob", bufs=4) as obpool:
            wo_sb = [wopool.tile([128, E], f32r, name=f"wo{c}", tag=f"wo{c}")
                     for c in range(4)]
            for c in range(4):
                nc.sync.dma_start(out=wo_sb[c],
                                  in_=wo[c * 128:(c + 1) * 128, :])
            for s in range(NST):
                for mb in range(8):
                    ps = ppool.tile([128, NQ], f32, tag="pp")
                    for cb in range(4):
                        nc.tensor.matmul(
                            ps[:, :],
                            lhsT=wo_sb[cb][:, mb * 128:(mb + 1) * 128],
                            rhs=attnT[cb][:, s * NQ:(s + 1) * NQ],
                            start=(cb == 0), stop=(cb == 3))
                    ob = obpool.tile([128, NQ], f32, tag="ob")
                    if mb % 2 == 0:
                        nc.scalar.copy(ob[:, :], ps[:, :])
                    else:
                        nc.vector.tensor_copy(ob[:, :], ps[:, :])
                    nc.sync.dma_start(
                        out=outT[mb * 128:(mb + 1) * 128,
                                 s * NQ:(s + 1) * NQ],
                        in_=ob[:, :])

    nc.compile()
    return nc


def _prep_inputs(x_padded, Wqkv, Wout, seq_lengths):
    """Per-core input maps."""
    Wq = Wqkv[0:E]
    Wk = Wqkv[E:2 * E]
    Wv = Wqkv[2 * E:3 * E]

    # static window mask tiles (identical for every core)
    p = np.arange(128)[:, None]
    f = np.arange(NQ)[None, :]
    m_a = np.where(f <= p, 0.0, NEG).astype(np.float32)
    m_b = np.where(f <= p + 128, 0.0, NEG).astype(np.float32)
    m_c = np.where(f >= p, 0.0, NEG).astype(np.float32)
    m_d = np.where(f >= p + 128, 0.0, NEG).astype(np.float32)
    ident_plane = np.zeros((128, 512), dtype=np.float32)
    ident_plane[:, 0:128] = np.eye(128, dtype=np.float32)
    masks = np.stack([np.concatenate([m_a, m_b], axis=1),
                      np.concatenate([m_c, m_d], axis=1),
                      ident_plane])

    in_maps = []
    for c in range(NCORES):
        b, g = divmod(c, 2)
        hs = np.arange(g * HPC, (g + 1) * HPC)
        rows = (hs[:, None] * D + np.arange(D)[None, :]).reshape(-1)
        wqk_c = np.concatenate([Wq[rows] * SCALE, Wk[rows]], axis=0)
        valid = (np.arange(S) < seq_lengths[b]).astype(np.float32)
        in_maps.append({
            "xT": np.ascontiguousarray(x_padded[b].T),
            "wqk": np.ascontiguousarray(wqk_c.T),
            "wv": np.ascontiguousarray(Wv[rows].T),
            "wo": np.ascontiguousarray(Wout[:, rows].T),
            "vmask": np.ascontiguousarray(valid.reshape(16, 128).T),
            "vone8": np.ascontiguousarray(
                np.repeat(valid[:, None], HPC, axis=1)),
            "masks": masks,
        })
    return in_maps


def _make_runner(nc):
    """Reusable jitted SPMD executor (the multi-core path of
    bass2jax.run_bass_via_pjrt, kept alive so repeat runs skip re-tracing)."""
    import jax
    import numpy as np
    from jax.experimental.shard_map import shard_map
    from jax.sharding import Mesh, PartitionSpec

    import concourse.mybir as mybir
    from concourse.bass2jax import (_bass_exec_p, install_neuronx_cc_hook,
                                    partition_id_tensor)

    install_neuronx_cc_hook()
    partition_name = (nc.partition_id_tensor.name
                      if nc.partition_id_tensor else None)
    in_names, out_names, out_avals, zero_outs = [], [], [], []
    for alloc in nc.m.functions[0].allocations:
        if not isinstance(alloc, mybir.MemoryLocationSet):
            continue
        name = alloc.memorylocations[0].name
        if alloc.kind == "ExternalInput":
            if name != partition_name:
                in_names.append(name)
        elif alloc.kind == "ExternalOutput":
            shape = tuple(alloc.tensor_shape)
            dtype = mybir.dt.np(alloc.dtype)
            out_names.append(name)
            out_avals.append(jax.core.ShapedArray(shape, dtype))
            zero_outs.append(np.zeros(shape, dtype))
    n_params = len(in_names)
    n_outs = len(out_avals)
    all_in_names = list(in_names) + list(out_names)
    if partition_name is not None:
        all_in_names.append(partition_name)
    donate = tuple(range(n_params, n_params + n_outs))

    def _body(*args):
        operands = list(args)
        if partition_name is not None:
            operands.append(partition_id_tensor())
        outs = _bass_exec_p.bind(
            *operands,
            out_avals=tuple(out_avals),
            in_names=tuple(all_in_names),
            out_names=tuple(out_names),
            lowering_input_output_aliases=(),
            sim_require_finite=True,
            sim_require_nnan=True,
            nc=nc,
        )
        return tuple(outs)

    devices = jax.devices()[:NCORES]
    mesh = Mesh(np.asarray(devices), ("core",))
    in_specs = (PartitionSpec("core"),) * (n_params + n_outs)
    out_specs = (PartitionSpec("core"),) * len(out_names)
    sharded = jax.jit(
        shard_map(_body, mesh=mesh, in_specs=in_specs, out_specs=out_specs,
                  check_rep=False),
        donate_argnums=donate, keep_unused=True)

    def prep(in_maps):
        concat_in = [
            np.concatenate([np.asarray(in_maps[c][nm]) for c in range(NCORES)],
                           axis=0)
            for nm in in_names]
        concat_zeros = [np.zeros((NCORES * z.shape[0], *z.shape[1:]), z.dtype)
                        for z in zero_outs]
        return concat_in, concat_zeros

    def run_prepped(concat_in, concat_zeros):
        return sharded(*concat_in, *concat_zeros)

    def run(in_maps):
        concat_in, concat_zeros = prep(in_maps)
        out_arrs = run_prepped(concat_in, concat_zeros)
        return [
            {nm: np.asarray(out_arrs[i]).reshape(NCORES, *out_avals[i].shape)[c]
             for i, nm in enumerate(out_names)}
            for c in range(NCORES)]

    run.prep = prep
    run.run_prepped = run_prepped
    return run


def get_runner():
    if "runner" not in _cache:
        if "nc" not in _cache:
            _cache["nc"] = _build_program()
        _cache["runner"] = _make_runner(_cache["nc"])
    return _cache["runner"]


def kernel(x_padded, Wqkv, Wout, seq_lengths, window_left, window_right):
    assert int(window_left) == WIN and int(window_right) == 0
    x_padded = np.asarray(x_padded, dtype=np.float32)
    Wqkv = np.asarray(Wqkv, dtype=np.float32)
    Wout = np.asarray(Wout, dtype=np.float32)
    seq_lengths = np.asarray(seq_lengths, dtype=np.int32)

    run = get_runner()
    in_maps = _prep_inputs(x_padded, Wqkv, Wout, seq_lengths)
    results = run(in_maps)

    out = np.empty((B, S, E), dtype=np.float32)
    for b in range(B):
        acc = results[2 * b]["outT"] + results[2 * b + 1]["outT"]
        out[b] = acc.T

    # fully-masked query rows: window [i-512, i] entirely past seq_len
    Wv = Wqkv[2 * E:3 * E]
    for b in range(B):
        sl = int(seq_lengths[b])
        if sl + WIN < S:
            v_mean = x_padded[b].mean(axis=0) @ Wv.T
            out[b, sl + WIN:, :] = v_mean @ Wout.T
    return out


# revision 9
# speedup vs baseline: 52.0422x; 52.0422x over previous
"""Sliding-window attention block (B=4, S=2048, E=1024, H=16, D=64,
window_left=512, window_right=0) on 8 Trainium2 NeuronCores.

Sharding: core c handles batch b=c//2 and head group g=c%2 (8 heads each).
Each core computes qkv projection for its heads over the full sequence,
banded attention (256-query stripes, 128-key blocks), and a partial output
projection; the host sums the two head-group partials per batch.

All device dataflow is feature-major (transposed): qkT/attnT/outT are
[features, seq].  Window masking is added into the scores PSUM with
identity-weight matmuls of precomputed -30000 bias tiles.  Key padding
(j >= seq_len) is handled by zeroing V rows and the denominator-ones
column, so padded keys drop out of both numerator and denominator.
Fully-masked query rows (i >= seq_len+512) are fixed up on the host
(reference semantics: uniform attention over all keys).
"""

import numpy as np

B, S, E, H, D = 4, 2048, 1024, 16, 64
NCORES = 8
HPC = H // 2          # heads per core
WIN = 512             # window_left (window_right = 0)
NEG = -30000.0
NQ = 256              # query stripe width
NST = S // NQ         # stripes
SCALE = 1.0 / np.sqrt(np.float32(D))

_cache = {}


def _build_program():
    from contextlib import ExitStack

    import concourse.bass as bass  # noqa: F401
    import concourse.mybir as mybir
    import concourse.tile as tile
    from concourse import bacc

    dt = mybir.dt
    f32, f32r = dt.float32, dt.float32r
    AF = mybir.ActivationFunctionType
    mult = mybir.AluOpType.mult

    nc = bacc.Bacc("TRN2", target_bir_lowering=False, debug=False,
                   num_devices=NCORES)

    xT = nc.dram_tensor("xT", [E, S], f32r, kind="ExternalInput")
    wqk = nc.dram_tensor("wqk", [E, 2 * HPC * D], f32r, kind="ExternalInput")
    wv = nc.dram_tensor("wv", [E, HPC * D], f32r, kind="ExternalInput")
    wo = nc.dram_tensor("wo", [HPC * D, E], f32r, kind="ExternalInput")
    vmask = nc.dram_tensor("vmask", [128, 16], f32, kind="ExternalInput")
    vone8 = nc.dram_tensor("vone8", [S, HPC], f32r, kind="ExternalInput")
    masks = nc.dram_tensor("masks", [3, 128, 512], f32r, kind="ExternalInput")
    outT = nc.dram_tensor("outT", [E, S], f32, kind="ExternalOutput")

    with tile.TileContext(nc) as tc, ExitStack() as ctx:
        persist = ctx.enter_context(tc.tile_pool(name="persist", bufs=1))

        qkT = [persist.tile([128, S], f32r, name=f"qkT{i}", tag=f"qkT{i}") for i in range(8)]
        vsb = [persist.tile([128, HPC, D + 1], f32r, name=f"v{t}", tag=f"v{t}")
               for t in range(16)]
        maskAB = persist.tile([128, 512], f32r, tag="maskAB")
        maskCD = persist.tile([128, 512], f32r, tag="maskCD")
        vmsb = persist.tile([128, 16], f32, tag="vmsb")
        ident = persist.tile([128, 128], f32r, tag="ident")

        nc.sync.dma_start(out=ident, in_=masks[2, :, 0:128])
        nc.sync.dma_start(out=maskAB, in_=masks[0])
        nc.sync.dma_start(out=maskCD, in_=masks[1])
        nc.sync.dma_start(out=vmsb, in_=vmask[:, :])

        # ---- phase 1+2: qk projection (feature-major) + V (seq-major) ----
        with tc.tile_pool(name="wgt12", bufs=1) as wpool, \
             tc.tile_pool(name="xc", bufs=2) as xpool, \
             tc.tile_pool(name="qkps", bufs=2, space="PSUM") as qkps, \
             tc.tile_pool(name="vps", bufs=2, space="PSUM") as vps:
            wqk_sb = [wpool.tile([128, 2 * HPC * D], f32r, name=f"wqk{k}",
                                 tag=f"wqk{k}") for k in range(8)]
            wv_sb = [wpool.tile([128, HPC * D], f32r, name=f"wv{k}",
                                tag=f"wv{k}") for k in range(8)]
            for k in range(8):
                nc.sync.dma_start(out=wqk_sb[k],
                                  in_=wqk[k * 128:(k + 1) * 128, :])
                nc.sync.dma_start(out=wv_sb[k],
                                  in_=wv[k * 128:(k + 1) * 128, :])
            for nb in range(4):
                xc = [xpool.tile([128, 512], f32r, name=f"xc{k}", tag=f"xc{k}")
                      for k in range(8)]
                for k in range(8):
                    nc.sync.dma_start(
                        out=xc[k],
                        in_=xT[k * 128:(k + 1) * 128, nb * 512:(nb + 1) * 512])
                for mb in range(8):
                    ps = qkps.tile([128, 512], f32, tag="qk")
                    for k in range(8):
                        nc.tensor.matmul(
                            ps[:, :],
                            lhsT=wqk_sb[k][:, mb * 128:(mb + 1) * 128],
                            rhs=xc[k][:, :],
                            start=(k == 0), stop=(k == 7))
                    dst = qkT[mb][:, nb * 512:(nb + 1) * 512]
                    if mb % 2 == 0:
                        nc.scalar.copy(dst, ps[:, :])
                    else:
                        nc.vector.tensor_copy(dst, ps[:, :])
                for t4 in range(4):
                    t = nb * 4 + t4
                    ps = vps.tile([128, 512], f32, tag="v")
                    for k in range(8):
                        nc.tensor.matmul(
                            ps[:, :],
                            lhsT=xc[k][:, t4 * 128:(t4 + 1) * 128],
                            rhs=wv_sb[k][:, :],
                            start=(k == 0), stop=(k == 7))
                    nc.vector.tensor_scalar(
                        out=vsb[t][:, :, 0:D],
                        in0=ps.rearrange("p (h d) -> p h d", h=HPC),
                        scalar1=vmsb[:, t:t + 1],
                        scalar2=None,
                        op0=mult)
                    nc.sync.dma_start(out=vsb[t][:, :, D],
                                      in_=vone8[t * 128:(t + 1) * 128, :])

        # ---- phase 3: banded attention ----
        apool = ctx.enter_context(tc.tile_pool(name="attn", bufs=1))
        attnT = [apool.tile([128, S], f32r, name=f"attnT{i}", tag=f"attnT{i}")
                 for i in range(4)]
        with tc.tile_pool(name="scps", bufs=2, space="PSUM") as spool, \
             tc.tile_pool(name="ops", bufs=2, space="PSUM") as opool, \
             tc.tile_pool(name="expT", bufs=3) as epool, \
             tc.tile_pool(name="rc", bufs=4) as rpool, \
             tc.tile_pool(name="rb", bufs=4) as rbpool:
            for s in range(NST):
                if s == 0:
                    kbs, mask_regions = [4, 5], [(0, maskCD)]
                elif s == 1:
                    kbs, mask_regions = [2, 3, 4, 5], [(2, maskCD)]
                else:
                    kbs, mask_regions = [0, 1, 2, 3, 4, 5], [(0, maskAB),
                                                             (4, maskCD)]
                nkb = len(kbs)
                base_kt = 2 * s - 4
                # per-PSUM-bank op lists: regions 2b, 2b+1 share a bank and
                # must form one accumulation group (start clears the bank)
                banks = []
                for b0 in range(0, nkb, 2):
                    ops = [("score", b0), ("score", b0 + 1)]
                    for reg0, mk in mask_regions:
                        if reg0 == b0:
                            ops += [("mask", (b0, 0, mk)), ("mask", (b0 + 1, 1, mk))]
                    banks.append(ops)
                for h in range(HPC):
                    po = (h % 2) * 64
                    qt = qkT[h // 2]
                    kt_ = qkT[4 + h // 2]
                    sc = spool.tile([128, 6, NQ], f32, tag="sc")
                    for ops in banks:
                        for oi, (kind, arg) in enumerate(ops):
                            first, last = oi == 0, oi == len(ops) - 1
                            if kind == "score":
                                ktile = base_kt + kbs[arg]
                                nc.tensor.matmul(
                                    sc[:, arg, :],
                                    lhsT=kt_[po:po + 64,
                                               ktile * 128:(ktile + 1) * 128],
                                    rhs=qt[po:po + 64, s * NQ:(s + 1) * NQ],
                                    start=first, stop=last)
                            else:
                                reg, j, mk = arg
                                nc.tensor.matmul(
                                    sc[:, reg, :],
                                    lhsT=ident[:, :],
                                    rhs=mk[:, j * 256:(j + 1) * 256],
                                    start=first, stop=last)
                    ex = epool.tile([128, 6, NQ], f32r, tag="ex")
                    nc.scalar.activation(ex[:, 0:nkb, :], sc[:, 0:nkb, :],
                                         AF.Exp)
                    ot = opool.tile([D + 1, NQ], f32, tag="ot")
                    for i, kb in enumerate(kbs):
                        ktile = base_kt + kb
                        nc.tensor.matmul(
                            ot[:, :],
                            lhsT=vsb[ktile][:, h, :],
                            rhs=ex[:, i, :],
                            start=(i == 0), stop=(i == nkb - 1))
                    rc = rpool.tile([1, NQ], f32, tag="rc")
                    nc.vector.reciprocal(rc[:, :], ot[D:D + 1, :])
                    rb = rbpool.tile([64, NQ], f32, tag="rb")
                    nc.gpsimd.partition_broadcast(rb[:, :], rc[:, :])
                    nc.vector.tensor_tensor(
                        out=attnT[h // 2][po:po + 64, s * NQ:(s + 1) * NQ],
                        in0=ot[0:D, :], in1=rb[:, :], op=mult)

        # ---- phase 4: output projection ----
        with tc.tile_pool(name="wo4", bufs=1) as wopool, \
             tc.tile_pool(name="oprj", bufs=4, space="PSUM") as ppool, \
             tc.tile_pool(name="ob", bufs=4) as obpool:
            wo_sb = [wopool.tile([128, E], f32r, name=f"wo{c}", tag=f"wo{c}")
                     for c in range(4)]
            for c in range(4):
                nc.sync.dma_start(out=wo_sb[c],
                                  in_=wo[c * 128:(c + 1) * 128, :])
            for s in range(NST):
                for mb in range(8):
                    ps = ppool.tile([128, NQ], f32, tag="pp")
                    for cb in range(4):
                        nc.tensor.matmul(
                            ps[:, :],
                            lhsT=wo_sb[cb][:, mb * 128:(mb + 1) * 128],
                            rhs=attnT[cb][:, s * NQ:(s + 1) * NQ],
                            start=(cb == 0), stop=(cb == 3))
                    ob = obpool.tile([128, NQ], f32, tag="ob")
                    if mb % 2 == 0:
                        nc.scalar.copy(ob[:, :], ps[:, :])
                    else:
                        nc.vector.tensor_copy(ob[:, :], ps[:, :])
                    nc.sync.dma_start(
                        out=outT[mb * 128:(mb + 1) * 128,
                                 s * NQ:(s + 1) * NQ],
                        in_=ob[:, :])

    nc.compile()
    return nc


def _prep_inputs(x_padded, Wqkv, Wout, seq_lengths):
    """Per-core input maps."""
    Wq = Wqkv[0:E]
    Wk = Wqkv[E:2 * E]
    Wv = Wqkv[2 * E:3 * E]

    # static window mask tiles (identical for every core)
    p = np.arange(128)[:, None]
    f = np.arange(NQ)[None, :]
    m_a = np.where(f <= p, 0.0, NEG).astype(np.float32)
    m_b = np.where(f <= p + 128, 0.0, NEG).astype(np.float32)
    m_c = np.where(f >= p, 0.0, NEG).astype(np.float32)
    m_d = np.where(f >= p + 128, 0.0, NEG).astype(np.float32)
    ident_plane = np.zeros((128, 512), dtype=np.float32)
    ident_plane[:, 0:128] = np.eye(128, dtype=np.float32)
    masks = np.stack([np.concatenate([m_a, m_b], axis=1),
                      np.concatenate([m_c, m_d], axis=1),
                      ident_plane])

    in_maps = []
    for c in range(NCORES):
        b, g = divmod(c, 2)
        hs = np.arange(g * HPC, (g + 1) * HPC)
        rows = (hs[:, None] * D + np.arange(D)[None, :]).reshape(-1)
        wqk_c = np.concatenate([Wq[rows] * SCALE, Wk[rows]], axis=0)
        valid = (np.arange(S) < seq_lengths[b]).astype(np.float32)
        in_maps.append({
            "xT": np.ascontiguousarray(x_padded[b].T),
            "wqk": np.ascontiguousarray(wqk_c.T),
            "wv": np.ascontiguousarray(Wv[rows].T),
            "wo": np.ascontiguousarray(Wout[:, rows].T),
            "vmask": np.ascontiguousarray(valid.reshape(16, 128).T),
            "vone8": np.ascontiguousarray(
                np.repeat(valid[:, None], HPC, axis=1)),
            "masks": masks,
        })
    return in_maps


def _make_runner(nc):
    """Reusable jitted SPMD executor (the multi-core path of
    bass2jax.run_bass_via_pjrt, kept alive so repeat runs skip re-tracing)."""
    import jax
    import numpy as np
    from jax.experimental.shard_map import shard_map
    from jax.sharding import Mesh, PartitionSpec

    import concourse.mybir as mybir
    from concourse.bass2jax import (_bass_exec_p, install_neuronx_cc_hook,
                                    partition_id_tensor)

    install_neuronx_cc_hook()
    partition_name = (nc.partition_id_tensor.name
                      if nc.partition_id_tensor else None)
    in_names, out_names, out_avals, zero_outs = [], [], [], []
    for alloc in nc.m.functions[0].allocations:
        if not isinstance(alloc, mybir.MemoryLocationSet):
            continue
        name = alloc.memorylocations[0].name
        if alloc.kind == "ExternalInput":
            if name != partition_name:
                in_names.append(name)
        elif alloc.kind == "ExternalOutput":
            shape = tuple(alloc.tensor_shape)
            dtype = mybir.dt.np(alloc.dtype)
            out_names.append(name)
            out_avals.append(jax.core.ShapedArray(shape, dtype))
            zero_outs.append(np.zeros(shape, dtype))
    n_params = len(in_names)
    n_outs = len(out_avals)
    all_in_names = list(in_names) + list(out_names)
    if partition_name is not None:
        all_in_names.append(partition_name)
    donate = tuple(range(n_params, n_params + n_outs))

    def _body(*args):
        operands = list(args)
        if partition_name is not None:
            operands.append(partition_id_tensor())
        outs = _bass_exec_p.bind(
            *operands,
            out_avals=tuple(out_avals),
            in_names=tuple(all_in_names),
            out_names=tuple(out_names),
            lowering_input_output_aliases=(),
            sim_require_finite=True,
            sim_require_nnan=True,
            nc=nc,
        )
        return tuple(outs)

    devices = jax.devices()[:NCORES]
    mesh = Mesh(np.asarray(devices), ("core",))
    in_specs = (PartitionSpec("core"),) * (n_params + n_outs)
    out_specs = (PartitionSpec("core"),) * len(out_names)
    sharded = jax.jit(
        shard_map(_body, mesh=mesh, in_specs=in_specs, out_specs=out_specs,
                  check_rep=False),
        donate_argnums=donate, keep_unused=True)

    def prep(in_maps):
        concat_in = [
            np.concatenate([np.asarray(in_maps[c][nm]) for c in range(NCORES)],
                           axis=0)
            for nm in in_names]
        concat_zeros = [np.zeros((NCORES * z.shape[0], *z.shape[1:]), z.dtype)
                        for z in zero_outs]
        return concat_in, concat_zeros

    def run_prepped(concat_in, concat_zeros):
        return sharded(*concat_in, *concat_zeros)

    def run(in_maps):
        concat_in, concat_zeros = prep(in_maps)
        out_arrs = run_prepped(concat_in, concat_zeros)
        return [
            {nm: np.asarray(out_arrs[i]).reshape(NCORES, *out_avals[i].shape)[c]
             for i, nm in enumerate(out_names)}
            for c in range(NCORES)]

    run.prep = prep
    run.run_prepped = run_prepped
    run.mesh = mesh
    return run


def get_runner():
    if "runner" not in _cache:
        if "nc" not in _cache:
            _cache["nc"] = _build_program()
        _cache["runner"] = _make_runner(_cache["nc"])
    return _cache["runner"]


def kernel(x_padded, Wqkv, Wout, seq_lengths, window_left, window_right):
    assert int(window_left) == WIN and int(window_right) == 0
    x_padded = np.asarray(x_padded, dtype=np.float32)
    Wqkv = np.asarray(Wqkv, dtype=np.float32)
    Wout = np.asarray(Wout, dtype=np.float32)
    seq_lengths = np.asarray(seq_lengths, dtype=np.int32)

    run = get_runner()
    in_maps = _prep_inputs(x_padded, Wqkv, Wout, seq_lengths)
    results = run(in_maps)

    out = np.empty((B, S, E), dtype=np.float32)
    for b in range(B):
        acc = results[2 * b]["outT"] + results[2 * b + 1]["outT"]
        out[b] = acc.T

    # fully-masked query rows: window [i-512, i] entirely past seq_len
    Wv = Wqkv[2 * E:3 * E]
    for b in range(B):
        sl = int(seq_lengths[b])
        if sl + WIN < S:
            v_mean = x_padded[b].mean(axis=0) @ Wv.T
            out[b, sl + WIN:, :] = v_mean @ Wout.T
    return out
